# revision 30
# baseline (speedup 1.0000x reference)
# Trainium2 Bass kernel for nn_MHA_18657337934739
#
# MHA: qkv = x@Wqkv + b; q,k = rope(q),rope(k); softmax(q k^T / 8) @ v; proj.
# Shapes: B=4, T=2048, C=1024, H=16 heads, D=64.
#
# Sharding: 8 cores = (4 batches) x (2 head-groups of 8 heads).  Each core
# computes its batch's attention for its 8 heads plus the partial output
# projection (contraction over its 512 local channels).  Host sums the two
# partials per batch (tensor-parallel unshard) and transposes back.
#
# On-core dataflow (all matmul inputs bf16, PSUM accumulation f32):
#   qk_T[c', t] = Wqk_loc^T x^T   (channels on partitions -> RoPE via
#                                  partition-pair swap DMA + cos/sin tables)
#   v[t, d]     = x Wv_loc        (tokens on partitions; +ones column)
#   S_T[s, q]   = K_rot^T Q_rot   (row-tiled pairs: two K=64 matmuls share
#                                  the PE array via tile_position rows 0/64)
#   P = exp(S_T / 8)              (ScalarE, no max-subtraction: |S|<~4)
#   O'[d+1, q]  = [V|1]^T P       (M=65: row 64 = softmax denominator)
#   attnT       = O'[0:64]/denom  (recip folded 64-wide + SBUF broadcast)
#   out_T       = Wproj_loc^T attnT + b  (partial; host sums group pairs)
#
# Schedule: the exp ACTIVATEs (256 x ~1.1us) are the critical resource.
# The emission software-pipelines QK one iteration ahead (QK(g+1) enters
# the tensor queue before aux work and PV(g)), so ScalarE runs exp
# back-to-back.  Input DMAs are chunk-split and priority-ordered across
# two rings (sync HWDGE + gpsimd SWDGE) so the first exp fires ~14us in.
# Pairs are processed 0,1 then 3/2 interleaved per chunk so the output
# projection spreads into loop slack instead of piling up at the tail.

import numpy as np
import ml_dtypes

import concourse.bass as bass
import concourse.tile as tile
from concourse import bacc, mybir
from concourse.bass_utils import run_bass_kernel_spmd

BF16 = mybir.dt.bfloat16
F32 = mybir.dt.float32

B, T, C = 4, 2048, 1024
H, D = 16, 64
ROPE_BASE = 10000.0
SCALE = 1.0 / 8.0  # 1/sqrt(D)

NCORES = 8
LH = 8          # local heads per core
PAIRS = LH // 2  # 4
CS = C // 128    # 8 contraction subtiles
TT = T // 128    # 16 token tiles
CH = 512         # q-chunk width
NCHUNK = T // CH  # 4
VW = D + 1       # 65: V plus ones column


def _bf16(a):
    return np.ascontiguousarray(a).astype(ml_dtypes.bfloat16)


def _f32(a):
    return np.ascontiguousarray(a).astype(np.float32)


def build_program():
    nc = bacc.Bacc("TRN2", target_bir_lowering=False, debug=False)

    # chunk-major x so one chunk's 8 contraction subtiles are one DMA
    xT = nc.dram_tensor("xT", [128, NCHUNK, CS, CH], BF16, kind="ExternalInput")
    # j-major qk weights so one head-pair tile is one DMA
    wqk = nc.dram_tensor("wqk", [128, 8, CS, 128], BF16, kind="ExternalInput")
    wv = nc.dram_tensor("wv", [128, CS, LH * D], BF16, kind="ExternalInput")
    wpo = nc.dram_tensor("wpo", [128, PAIRS, 1024], BF16, kind="ExternalInput")
    bqk = nc.dram_tensor("bqk", [128, 8], F32, kind="ExternalInput")
    bvbc = nc.dram_tensor("bvbc", [128, LH * D], F32, kind="ExternalInput")
    bpo = nc.dram_tensor("bpo", [128, 8], F32, kind="ExternalInput")
    cosb = nc.dram_tensor("cosb", [128, NCHUNK, CH], BF16, kind="ExternalInput")
    sinb = nc.dram_tensor("sinb", [128, NCHUNK, CH], BF16, kind="ExternalInput")
    perm = nc.dram_tensor("perm", [128, 128], BF16, kind="ExternalInput")
    outT = nc.dram_tensor("outT", [128, 8, T], BF16, kind="ExternalOutput")

    with tile.TileContext(nc) as tc:
        with (
            tc.tile_pool(name="sb", bufs=1) as sb,
            tc.tile_pool(name="work", bufs=2) as work,
            tc.tile_pool(name="dsc", bufs=4, space="DRAM") as dsc,
            tc.tile_pool(name="pp", bufs=2, space="PSUM") as pp,
            tc.tile_pool(name="qkp", bufs=2, space="PSUM") as qkp,
            tc.tile_pool(name="avp", bufs=2, space="PSUM") as avp,
        ):
            # ---- resident SBUF tensors ----
            xT_sb = sb.tile([128, NCHUNK, CS, CH], BF16, name="xT_sb")
            wqk_sb = sb.tile([128, 8, CS, 128], BF16, name="wqk_sb")
            wv_sb = sb.tile([128, CS, LH * D], BF16, name="wv_sb")
            wpo_sb = sb.tile([128, PAIRS, 1024], BF16, name="wpo_sb")
            bqk_sb = sb.tile([128, 8], F32, name="bqk_sb")
            bv_sb = sb.tile([128, LH * D], F32, name="bv_sb")
            bpo_sb = sb.tile([128, 8], F32, name="bpo_sb")
            cos_sb = sb.tile([128, T], BF16, name="cos_sb")
            sin_sb = sb.tile([128, T], BF16, name="sin_sb")
            vv = sb.tile([128, TT, LH * VW], BF16, name="vv")
            qkr = [sb.tile([128, T], BF16, name=f"qkr{j}") for j in range(8)]
            attnT = [sb.tile([128, T], BF16, name=f"attnT{p}") for p in range(PAIRS)]

            SD = nc.sync.dma_start
            GD = nc.gpsimd.dma_start

            # ---- input DMAs, priority-ordered across the two rings ----
            # sync ring: what the first QK / first PV needs, then the rest
            perm_sb = sb.tile([128, 128], BF16, name="perm_sb")
            SD(out=bqk_sb[:], in_=bqk[:])
            SD(out=perm_sb[:], in_=perm[:])
            SD(out=wqk_sb[:, 4], in_=wqk[:, 4])
            SD(out=xT_sb[:, 0], in_=xT[:, 0])
            SD(out=wqk_sb[:, 0], in_=wqk[:, 0])
            SD(out=wv_sb[:], in_=wv[:])
            SD(out=bv_sb[:], in_=bvbc[:])
            SD(out=xT_sb[:, 1], in_=xT[:, 1])
            SD(out=wqk_sb[:, 5], in_=wqk[:, 5])
            SD(out=wqk_sb[:, 1], in_=wqk[:, 1])
            SD(out=xT_sb[:, 2], in_=xT[:, 2])
            SD(out=xT_sb[:, 3], in_=xT[:, 3])
            for j in (6, 2, 7, 3):
                SD(out=wqk_sb[:, j], in_=wqk[:, j])
            for c in range(1, NCHUNK):
                SD(out=cos_sb[:, c * CH : (c + 1) * CH], in_=cosb[:, c])
                SD(out=sin_sb[:, c * CH : (c + 1) * CH], in_=sinb[:, c])
            SD(out=wpo_sb[:], in_=wpo[:])
            SD(out=bpo_sb[:], in_=bpo[:])
            # gpsimd ring: only chunk-0 rope tables, so the sync ring's
            # critical xT/wqk stream keeps the bandwidth
            GD(out=cos_sb[:, 0:CH], in_=cosb[:, 0])
            GD(out=sin_sb[:, 0:CH], in_=sinb[:, 0])

            # ones column of [V|1]
            ones_view = vv.rearrange("p t (h e) -> p t h e", e=VW)[:, :, :, D : D + 1]
            nc.vector.memset(ones_view, 1.0)

            # PE prewarm: dummy matmuls run during the input DMAs, releasing
            # the HAM clock throttle (4/8 -> 8/8) before real work lands
            warm = sb.tile([128, CH], BF16, name="warm")
            nc.vector.memset(warm[:, :], 0.0)
            wps = pp.tile([128, CH], F32, name="wps", tag="pj")
            for _ in range(24):
                nc.tensor.matmul(
                    wps[:, :], lhsT=warm[:, 0:128], rhs=warm[:, :],
                    start=True, stop=True,
                )

            _qk_stage = {}

            def qkproj_mm(j, c, pj, lo, hi):
                for cs in range(lo, hi):
                    nc.tensor.matmul(
                        pj[:, :],
                        lhsT=wqk_sb[:, j, cs, :],
                        rhs=xT_sb[:, c, cs, :],
                        start=(cs == 0),
                        stop=(cs == CS - 1),
                    )

            def qkproj_rope_finish(j, c, pj):
                """Bias + RoPE (partner swap is a 32-partition block
                exchange; head dims packed [evens|odds] on host)."""
                qp, sw = _qk_stage[j]
                cols = slice(c * CH, (c + 1) * CH)
                nc.vector.tensor_scalar_add(
                    qp[:, cols], pj[:, :], bqk_sb[:, j : j + 1]
                )
                # RoPE partner swap = permutation matmul on the (always
                # warm) PE; no DMA-ring latency in the rope chain
                swp = pp.tile([128, CH], F32, name="swp", tag="pj")
                nc.tensor.matmul(
                    swp[:, :], lhsT=perm_sb[:, :], rhs=qp[:, cols],
                    start=True, stop=True,
                )
                nc.vector.tensor_mul(qp[:, cols], qp[:, cols], cos_sb[:, cols])
                nc.vector.tensor_mul(sw[:, cols], swp[:, :], sin_sb[:, cols])
                nc.vector.tensor_add(qkr[j][:, cols], qp[:, cols], sw[:, cols])

            def qkproj_rope_chunk(j, c):
                """Full rotated qk_T tile j columns [c*CH, (c+1)*CH).
                j 0-3: Q pairs, 4-7: K pairs."""
                if j not in _qk_stage:
                    _qk_stage[j] = (
                        work.tile([128, T], BF16, name=f"qp{j}", tag="qp", bufs=3),
                        work.tile([128, T], BF16, name=f"sw{j}", tag="sw", bufs=3),
                    )
                pj = pp.tile([128, CH], F32, name="pj", tag="pj")
                qkproj_mm(j, c, pj, 0, CS)
                qkproj_rope_finish(j, c, pj)

            def prep_halves(j, c):
                """qkproj_rope_chunk split into two emission granules so a
                single aux blob never delays the pipelined QK matmul."""
                if j not in _qk_stage:
                    _qk_stage[j] = (
                        work.tile([128, T], BF16, name=f"qp{j}", tag="qp", bufs=3),
                        work.tile([128, T], BF16, name=f"sw{j}", tag="sw", bufs=3),
                    )
                state = {}

                def first():
                    state["pj"] = pp.tile([128, CH], F32, name="pj", tag="pj")
                    qkproj_mm(j, c, state["pj"], 0, CS // 2)

                def second():
                    qkproj_mm(j, c, state["pj"], CS // 2, CS)
                    qkproj_rope_finish(j, c, state["pj"])

                return first, second

            def vproj_mm(t, pj, lo, hi):
                for cs in range(lo, hi):
                    nc.tensor.matmul(
                        pj[:, :],
                        lhsT=xT_sb[:, t // 4, cs, (t % 4) * 128 : (t % 4 + 1) * 128],
                        rhs=wv_sb[:, cs, :],
                        start=(cs == 0),
                        stop=(cs == CS - 1),
                    )

            def vproj_finish(t, pj):
                src = pj.rearrange("p (h e) -> p h e", e=D)
                dst = vv[:, t, :].rearrange("p (h e) -> p h e", e=VW)[:, :, 0:D]
                badd = bv_sb.rearrange("p (h e) -> p h e", e=D)
                nc.vector.tensor_add(dst, src, badd)

            def vproj_halves(t):
                state = {}

                def first():
                    state["pj"] = pp.tile([128, LH * D], F32, name="pj", tag="pj")
                    vproj_mm(t, state["pj"], 0, CS // 2)

                def second():
                    vproj_mm(t, state["pj"], CS // 2, CS)
                    vproj_finish(t, state["pj"])

                return first, second

            def vproj_tile(t):
                pj = pp.tile([128, LH * D], F32, name="pj", tag="pj")
                vproj_mm(t, pj, 0, CS)
                vproj_finish(t, pj)

            ones_row = sb.tile([1, 64], BF16, name="ones_row")
            nc.vector.memset(ones_row[:, :], 1.0)

            def normalize(p, c, pv0, pv1, pe_bcast=False):
                """attnT[p][:, chunk c] = O'[0:64]/denom.

                Denominator row (partition D of pv) is folded onto 64
                partitions by an SBUF->SBUF DMA for the exact reciprocal,
                then broadcast back along partitions: mid-kernel via a DRAM
                row (latency hides), at the tail via a K=1 TensorE matmul
                (ones ⊗ row) so the idle PE does it with no DRAM latency."""
                cols = slice(c * CH, (c + 1) * CH)
                # h1 first at the tail: its chain is longer (partition-
                # shift DMA) and gates the output projection
                order = ((1, pv1), (0, pv0)) if pe_bcast else ((0, pv0), (1, pv1))
                for h, pv in order:
                    st = work.tile([VW, CH], F32, name=f"st{h}", tag=f"st{h}")
                    nc.vector.tensor_copy(st[:, :], pv[:, :])
                    dv = work.tile([64, CH // 64], F32, name=f"dv{h}", tag=f"dv{h}")
                    GD(out=dv[:, :], in_=st[D : D + 1, :])
                    nc.vector.reciprocal(dv[:, :], dv[:, :])
                    if pe_bcast:
                        # gpsimd DMA casts f32->bf16 so the broadcast
                        # matmul runs single-pass (fp32 rhs is multi-pass)
                        rrow = work.tile([1, CH], BF16, name=f"rw{h}", tag=f"rw{h}")
                        GD(out=rrow[:, :], in_=dv[:, :])
                        bc = pp.tile([64, CH], F32, name=f"bcp{h}", tag="pj")
                        nc.tensor.matmul(
                            bc[:, :], lhsT=ones_row[:, :], rhs=rrow[:, :],
                            start=True, stop=True,
                        )
                    else:
                        dn = dsc.tile([1, CH], F32, name=f"dn{h}", tag=f"dn{h}")
                        GD(out=dn[:, :], in_=dv[:, :])
                        bc = work.tile([64, CH], F32, name=f"bc{h}", tag=f"bc{h}")
                        GD(out=bc[:, :], in_=dn.to_broadcast([64, CH]))
                    if h == 0:
                        nc.vector.tensor_mul(
                            attnT[p][0:64, cols], st[0:64, :], bc[:, :]
                        )
                    else:
                        s1 = work.tile([64, CH], BF16, name="s1", tag="s1")
                        nc.vector.tensor_mul(s1[:, :], st[0:64, :], bc[:, :])
                        # at the tail, keep the shift off the gpsimd ring so
                        # its slow SWDGE drain starts early and overlaps
                        (SD if pe_bcast else GD)(
                            out=attnT[p][64:128, cols], in_=s1[:, :]
                        )

            def oproj_mm(c, j, pj, lo, hi):
                for p in range(lo, hi):
                    nc.tensor.matmul(
                        pj[:, :],
                        lhsT=wpo_sb[:, p, j * 128 : (j + 1) * 128],
                        rhs=attnT[p][:, c * CH : (c + 1) * CH],
                        start=(p == 0),
                        stop=(p == PAIRS - 1),
                    )

            def oproj_finish(c, j, pj):
                ob = work.tile([128, CH], BF16, name="ob", tag="ob", bufs=3)
                nc.vector.tensor_scalar_add(
                    ob[:, :], pj[:, :], bpo_sb[:, j : j + 1]
                )
                SD(out=outT[:, j, c * CH : (c + 1) * CH], in_=ob[:, :])

            def oproj_halves(c, j):
                state = {}

                def first():
                    state["pj"] = pp.tile([128, CH], F32, name="pj", tag="pj")
                    oproj_mm(c, j, state["pj"], 0, PAIRS // 2)

                def second():
                    oproj_mm(c, j, state["pj"], PAIRS // 2, PAIRS)
                    oproj_finish(c, j, state["pj"])

                return first, second

            # tail chunk: pre-contract pairs {0,1,3} into bf16 partials
            # during the last chunk's slack; the tail then needs only the
            # pair-2 matmul plus one fused add per piece
            opart = sb.tile([128, 8, CH], BF16, name="opart")

            def oproj_partial(c, j):
                def emit():
                    pj = pp.tile([128, CH], F32, name="pj", tag="pj")
                    for i, p in enumerate((0, 1)):
                        nc.tensor.matmul(
                            pj[:, :],
                            lhsT=wpo_sb[:, p, j * 128 : (j + 1) * 128],
                            rhs=attnT[p][:, c * CH : (c + 1) * CH],
                            start=(i == 0),
                            stop=(i == 1),
                        )
                    nc.vector.tensor_copy(opart[:, j, :], pj[:, :])
                return emit

            def oproj_final(c, j):
                def emit():
                    pj = pp.tile([128, CH], F32, name="pj", tag="pj")
                    for i, p in enumerate((3, 2)):
                        nc.tensor.matmul(
                            pj[:, :],
                            lhsT=wpo_sb[:, p, j * 128 : (j + 1) * 128],
                            rhs=attnT[p][:, c * CH : (c + 1) * CH],
                            start=(i == 0),
                            stop=(i == 1),
                        )
                    ob = work.tile([128, CH], BF16, name="ob", tag="ob", bufs=3)
                    nc.vector.scalar_tensor_tensor(
                        out=ob[:, :],
                        in0=pj[:, :],
                        scalar=bpo_sb[:, j : j + 1],
                        in1=opart[:, j, :],
                        op0=mybir.AluOpType.add,
                        op1=mybir.AluOpType.add,
                    )
                    SD(out=outT[:, j, c * CH : (c + 1) * CH], in_=ob[:, :])
                return emit

            # ---- global iteration plan ----
            # Pairs 0,1 chunk-sequential; then pairs 3,2 interleaved per
            # chunk so oproj(c) (which needs ALL pairs' attnT at chunk c)
            # can spread over the two following half-stretches.
            seq = [(0, 0), (0, 1), (0, 2), (0, 3),
                   (1, 0), (1, 1), (1, 2), (1, 3),
                   (3, 0), (2, 0), (3, 1), (2, 1),
                   (3, 2), (2, 2), (3, 3), (2, 3)]
            iters = [(p, c, s) for (p, c) in seq for s in range(TT)]
            NG = len(iters)

            def gi(p, c, s):
                return seq.index((p, c)) * TT + s

            sched = {}

            def put(g, fn):
                sched.setdefault(g, []).append(fn)

            def put_halves(g, halves):
                a, b = halves
                put(g, a)
                put(g + 1, b)

            # pair-0 chunk-0: all 16 V tiles (tile t by iteration t) plus
            # the remaining K0/Q0 chunks.  vproj t's second half lands at
            # iteration t (just before PV consumes vv[t]); K-chunk cc must
            # be fully emitted before iteration 4cc-1 (QK is pipelined one
            # iteration ahead).
            put(gi(0, 0, 0), lambda: vproj_tile(0))
            for t in range(1, TT):
                put_halves(gi(0, 0, t - 1), vproj_halves(t))
            put_halves(gi(0, 0, 1), prep_halves(4, 1))
            put_halves(gi(0, 0, 5), prep_halves(4, 2))
            put_halves(gi(0, 0, 9), prep_halves(4, 3))
            put_halves(gi(0, 0, 12), prep_halves(0, 1))
            # remaining preps, balanced at <=3 pieces per chunk; each is
            # emitted comfortably before its consuming QK's emission
            put_halves(gi(0, 1, 1), prep_halves(0, 2))
            put_halves(gi(0, 1, 5), prep_halves(5, 0))
            put_halves(gi(0, 1, 9), prep_halves(5, 1))
            put_halves(gi(0, 2, 1), prep_halves(5, 2))
            put_halves(gi(0, 2, 5), prep_halves(5, 3))
            put_halves(gi(0, 2, 9), prep_halves(0, 3))
            put_halves(gi(0, 3, 1), prep_halves(1, 0))
            put_halves(gi(0, 3, 5), prep_halves(1, 1))
            put_halves(gi(1, 0, 1), prep_halves(1, 2))
            put_halves(gi(1, 0, 5), prep_halves(1, 3))
            put_halves(gi(1, 1, 1), prep_halves(7, 0))
            put_halves(gi(1, 1, 5), prep_halves(7, 1))
            put_halves(gi(1, 1, 9), prep_halves(7, 2))
            put_halves(gi(1, 2, 1), prep_halves(7, 3))
            put_halves(gi(1, 2, 5), prep_halves(3, 0))
            put_halves(gi(1, 2, 9), prep_halves(3, 1))
            put_halves(gi(1, 3, 1), prep_halves(3, 2))
            put_halves(gi(1, 3, 5), prep_halves(3, 3))
            put_halves(gi(1, 3, 9), prep_halves(6, 0))
            put_halves(gi(3, 0, 1), prep_halves(6, 1))
            put_halves(gi(3, 0, 5), prep_halves(6, 2))
            put_halves(gi(3, 0, 9), prep_halves(6, 3))
            put_halves(gi(3, 0, 11), prep_halves(2, 0))
            put_halves(gi(2, 0, 1), prep_halves(2, 1))
            put_halves(gi(2, 0, 5), prep_halves(2, 2))
            put_halves(gi(2, 0, 9), prep_halves(2, 3))
            # oproj(c): first 4 pieces in pair-3's next chunk, last 4 in
            # pair-2's; chunk-3 pieces go to the tail
            # slots start at s=3: normalize(2,cc) is emitted two iterations
            # into the following chunk (PV lag), and these read its attnT
            for cc, (slot_a, slot_b) in enumerate(
                [((3, 1), (2, 1)), ((3, 2), (2, 2)), ((3, 3), (2, 3))]
            ):
                for j in range(4):
                    put_halves(gi(*slot_a, 2 * j + 3), oproj_halves(cc, j))
                    put_halves(gi(*slot_b, 2 * j + 3), oproj_halves(cc, 4 + j))
            # chunk-3 pair-{0,1} partials: attnT[0/1] chunk 3 is final once
            # normalize(1,3) lands (emission (3,0,1)); spread them over the
            # s=11,13 slots the oproj pieces leave free
            for i, j in enumerate(range(8)):
                slot = [(3, 1), (2, 1), (3, 2), (2, 2)][i // 2]
                put(gi(*slot, 11 + 2 * (i % 2)), oproj_partial(NCHUNK - 1, j))
            tail = [oproj_final(NCHUNK - 1, j) for j in range(8)]

            # ---- emission: software-pipelined attention ----
            sq_tiles = {}

            def emit_qk(g):
                p, c, s = iters[g]
                sq = qkp.tile([128, 2, CH], F32, name="sq", tag="sq")
                sq_tiles[g] = sq
                kt, qt = qkr[4 + p], qkr[p]
                for h in (0, 1):
                    nc.tensor.matmul(
                        sq[:, h, :],
                        lhsT=kt[h * 64 : (h + 1) * 64, s * 128 : (s + 1) * 128],
                        rhs=qt[h * 64 : (h + 1) * 64, c * CH : (c + 1) * CH],
                        start=True,
                        stop=True,
                    )

            # prologue: rope K0/Q0 chunk 0, then the first QK
            qkproj_rope_chunk(4, 0)
            qkproj_rope_chunk(0, 0)
            emit_qk(0)

            # PV emission lags two iterations so a boundary-stalled PV
            # (waiting the previous chunk's pv-copy WAR) sits behind the
            # next chunk's QKs in the in-order tensor stream instead of
            # blocking them.
            pv_tiles = {}
            ex_tiles = {}

            def emit_pv(g):
                p, c, s = iters[g]
                if s == 0:
                    pv_tiles[(p, c)] = (
                        avp.tile([VW, CH], F32, name="pv0", tag="pv"),
                        avp.tile([VW, CH], F32, name="pv1", tag="pv"),
                    )
                pv0, pv1 = pv_tiles[(p, c)]
                ex = ex_tiles.pop(g)
                for h, pv in ((0, pv0), (1, pv1)):
                    lh = 2 * p + h
                    nc.tensor.matmul(
                        pv[:, :],
                        lhsT=vv[:, s, lh * VW : lh * VW + VW],
                        rhs=ex[:, h, :],
                        start=(s == 0),
                        stop=(s == TT - 1),
                    )
                if s == TT - 1:
                    pv0, pv1 = pv_tiles.pop((p, c))
                    normalize(p, c, pv0, pv1, pe_bcast=((p, c) == seq[-1]))

            PVLAG = 2
            for g, (p, c, s) in enumerate(iters):
                if g + 1 < NG:
                    emit_qk(g + 1)
                for fn in sched.pop(g, ()):
                    fn()
                sq = sq_tiles.pop(g)
                ex = work.tile([128, 2, CH], BF16, name="ex", tag="ex", bufs=4)
                ex_tiles[g] = ex
                nc.scalar.activation(
                    out=ex[:, :, :],
                    in_=sq[:, :, :],
                    func=mybir.ActivationFunctionType.Exp,
                    scale=SCALE,
                )
                if g >= PVLAG:
                    emit_pv(g - PVLAG)
            for g in range(NG - PVLAG, NG):
                emit_pv(g)
            for fn in tail:
                fn()

    nc.compile()
    return nc


def rope_tables():
    """cos / sign-folded sin tables in [128 partitions, NCHUNK, CH] layout.

    Head dims are packed [evens | odds]: rows 0-31 hold x0 of pair k=row
    (sinsg = -sin), rows 32-63 hold x1 of pair k=row-32 (sinsg = +sin)."""
    k = np.arange(32).astype(np.float64)
    freqs = ROPE_BASE ** (-2.0 * k / D)  # [32]
    t = np.arange(T, dtype=np.float64)
    theta = t[None, :] * freqs[:, None]  # [32, T]
    cos64 = np.concatenate([np.cos(theta), np.cos(theta)], axis=0)
    sin64 = np.concatenate([-np.sin(theta), np.sin(theta)], axis=0)
    cos128 = np.tile(cos64, (2, 1)).reshape(128, NCHUNK, CH)
    sin128 = np.tile(sin64, (2, 1)).reshape(128, NCHUNK, CH)
    return _bf16(cos128), _bf16(sin128)


def swap_perm():
    """Symmetric 0/1 matrix: out = P^T @ qp exchanges 32-partition blocks
    0<->1 and 2<->3 (the RoPE partner swap in [evens|odds] packing)."""
    P = np.zeros((128, 128), dtype=np.float64)
    for base in (0, 64):
        for i in range(32):
            P[base + 32 + i, base + i] = 1.0
            P[base + i, base + 32 + i] = 1.0
    return _bf16(P)


def pack_group_weights(w_qkv, b_qkv, w_proj, b_proj, g):
    """Per-head-group weight shards in device layout."""
    Wq, Wk, Wv = w_qkv[:, :C], w_qkv[:, C : 2 * C], w_qkv[:, 2 * C :]
    bq, bk, bv = b_qkv[:C], b_qkv[C : 2 * C], b_qkv[2 * C :]
    heads = np.arange(g * LH, (g + 1) * LH)

    # qk tiles: j 0-3 = Q pairs, 4-7 = K pairs; each tile = 2 heads x 64 dims.
    # Within each head the dims are permuted [evens | odds] so the RoPE pair
    # partner is a 32-partition block swap (QK^T invariant to shared perm).
    eo = np.concatenate([np.arange(0, D, 2), np.arange(1, D, 2)])
    qk_cols = []
    bqk_cols = []
    for src, bias in ((Wq, bq), (Wk, bk)):
        for p in range(PAIRS):
            cols = np.concatenate(
                [heads[2 * p] * D + eo, heads[2 * p + 1] * D + eo]
            )
            qk_cols.append(src[:, cols])
            bqk_cols.append(bias[cols])
    wqk_l = np.concatenate(qk_cols, axis=1)  # [C, 1024]
    # j-major: [128, j, cs, 128]
    wqk_dev = _bf16(wqk_l.reshape(CS, 128, 8, 128).transpose(1, 2, 0, 3))
    bqk_dev = _f32(np.stack(bqk_cols, axis=1))  # [128, 8]

    vcols = np.concatenate([np.arange(h * D, h * D + D) for h in heads])
    wv_dev = _bf16(Wv[:, vcols].reshape(CS, 128, LH * D).transpose(1, 0, 2))
    bv_dev = _f32(np.broadcast_to(bv[vcols], (128, LH * D)))

    # proj rows in attnT order: local index p*128 + e*64 + d <-> head 2p+e
    rows = np.concatenate(
        [np.arange(heads[i] * D, heads[i] * D + D) for i in range(LH)]
    )
    wpo_l = w_proj[rows, :]  # [512, 1024]
    wpo_dev = _bf16(wpo_l.reshape(PAIRS, 128, 1024).transpose(1, 0, 2))

    bpo_full = b_proj if g == 0 else np.zeros_like(b_proj)  # avoid double bias
    bpo_dev = _f32(bpo_full.reshape(8, 128).T)

    return dict(wqk=wqk_dev, bqk=bqk_dev, wv=wv_dev, bvbc=bv_dev,
                wpo=wpo_dev, bpo=bpo_dev)


def make_in_maps(x, w_qkv, b_qkv, w_proj, b_proj):
    x = np.asarray(x, dtype=np.float32)
    w_qkv = np.asarray(w_qkv, dtype=np.float32)
    b_qkv = np.asarray(b_qkv, dtype=np.float32)
    w_proj = np.asarray(w_proj, dtype=np.float32)
    b_proj = np.asarray(b_proj, dtype=np.float32)

    cos_dev, sin_dev = rope_tables()
    perm_dev = swap_perm()
    gw = [pack_group_weights(w_qkv, b_qkv, w_proj, b_proj, g) for g in (0, 1)]

    in_maps = []
    for core in range(NCORES):
        b, g = core // 2, core % 2
        # chunk-major: [128, NCHUNK, CS, CH]
        xT_dev = _bf16(
            x[b].T.reshape(CS, 128, NCHUNK, CH).transpose(1, 2, 0, 3)
        )
        m = dict(xT=xT_dev, cosb=cos_dev, sinb=sin_dev, perm=perm_dev,
                 **gw[g])
        in_maps.append(m)
    return in_maps


_NC_CACHE = []


def get_nc():
    if not _NC_CACHE:
        _NC_CACHE.append(build_program())
    return _NC_CACHE[0]


def unshard(results):
    out = np.empty((B, T, C), dtype=np.float32)
    for b in range(B):
        acc = results[2 * b]["outT"].astype(np.float32) + results[2 * b + 1][
            "outT"
        ].astype(np.float32)
        out[b] = acc.transpose(1, 0, 2).reshape(C, T).T
    return out


def run(trace=False, **inputs):
    nc = get_nc()
    in_maps = make_in_maps(**inputs)
    res = run_bass_kernel_spmd(nc, in_maps, core_ids=list(range(NCORES)), trace=trace)
    return unshard(res.results), res


def kernel(**inputs) -> np.ndarray:
    out, _ = run(trace=False, **inputs)
    return out
